# revision 19
# baseline (speedup 1.0000x reference)
"""Trainium2 Bass kernel for nn_DeletionLayer (out = where(mask, x @ W, x)).

Strategy (data-parallel over the node dim N, 8 cores):
  - Host: transpose each core's x shard to xT [DIM, SHARD] so the matmul
    contraction dim (DIM) lands on SBUF partitions for BOTH operands with
    plain contiguous DMAs (no on-device transposes).
  - Device per core: psum[n] = (x @ W).T chunk via lhsT = W[k,n] chunks
    (stationary) x rhs = xT[k, rows] (moving, rounded to float32r on ACT);
    select: GpSimd copies the exact-f32 xT tile into the out tile, DVE
    overwrites masked columns from PSUM; DMA the tiles out as outT.
  - Host: transpose outT shards back and concatenate.
"""

import contextlib
import os
import sys

import numpy as np

for _p in ("/opt/trn_rl_repo", "/opt/pypackages"):
    if os.path.isdir(_p) and _p not in sys.path:
        sys.path.append(_p)

N_TOTAL = 131072
DIM = 512
N_CORES = 8
SHARD = N_TOTAL // N_CORES  # 16384

P = 128                      # SBUF partitions
KC = DIM // P                # 4 contraction chunks
ROWS_PER_BLOCK = int(os.environ.get("DK_RPB", "1024"))
HALF = ROWS_PER_BLOCK // 512 # psum banks per n-chunk (moving operand <= 512 fp32)
NBLK = SHARD // ROWS_PER_BLOCK
PSUM_BUFS = int(os.environ.get("DK_PSUM_BUFS", "1"))
XP_BUFS = int(os.environ.get("DK_XP_BUFS", "3"))
XRP_BUFS = int(os.environ.get("DK_XRP_BUFS", "2"))
OP_BUFS = int(os.environ.get("DK_OP_BUFS", "3"))

MM_DTYPE = "f32r"            # "f32r" (1 cyc/row) or "f32" (4 cyc/row, exact)
REPEAT = int(os.environ.get("DK_REPEAT", "1"))

_compiled = None


def _emit_block(nc, mm_dt, mybir, b, xt, out, w_tiles, mt, xp, xrp, op, pp):
    c0 = b * ROWS_PER_BLOCK
    cols = slice(c0, c0 + ROWS_PER_BLOCK)
    xts = []
    xrs = []
    for k in range(KC):
        xtile = xp.tile([P, ROWS_PER_BLOCK], mybir.dt.float32, tag=f"x{k}")
        nc.sync.dma_start(out=xtile, in_=xt[k * P:(k + 1) * P, cols])
        xts.append(xtile)
    for k in range(KC):
        xr = xrp.tile([P, ROWS_PER_BLOCK], mm_dt, tag=f"xr{k}")
        nc.scalar.copy(out=xr, in_=xts[k])  # rounds f32 -> f32r on ACT
        xrs.append(xr)

    for n in range(KC):
        pss = []
        for h in range(HALF):
            ps = pp.tile([P, 512], mybir.dt.float32, tag=f"ps{n}_{h}")
            pss.append(ps)
        for k in range(KC):
            lhsT = w_tiles[k][:, n * P:(n + 1) * P].bitcast(mm_dt)
            for h in range(HALF):
                nc.tensor.matmul(
                    pss[h],
                    lhsT,
                    xrs[k][:, h * 512:(h + 1) * 512],
                    start=(k == 0),
                    stop=(k == KC - 1),
                )
        otile = op.tile([P, ROWS_PER_BLOCK], mybir.dt.float32, tag=f"o{n}")
        # out = where(mask, (xW).T, xT): base copy on GpSimd, masked
        # overwrite from PSUM on DVE.
        nc.gpsimd.tensor_copy(out=otile, in_=xts[n])
        for h in range(HALF):
            sl = slice(h * 512, (h + 1) * 512)
            nc.vector.copy_predicated(
                out=otile[:, sl],
                mask=mt[:, c0 + h * 512:c0 + (h + 1) * 512],
                data=pss[h],
            )
        # out-DMA from the ACT HWDGE queue so it can't head-of-line block
        # the SP queue's input stream
        nc.scalar.dma_start(out=out[n * P:(n + 1) * P, cols], in_=otile)


def _build():
    import concourse.mybir as mybir
    from concourse import bacc
    from concourse.tile import TileContext

    mm_dt = mybir.dt.float32r if MM_DTYPE == "f32r" else mybir.dt.float32

    nc = bacc.Bacc(trn_type="TRN2")
    # x stays float32 end-to-end on the passthrough path (bit-exact where
    # mask is false); a rounded float32r copy is made on-device for the
    # matmul. w is only ever a matmul operand, so it can live as f32r.
    xt = nc.dram_tensor("xt", [DIM, SHARD], mybir.dt.float32,
                        kind="ExternalInput").ap()
    mask = nc.dram_tensor("mask", [1, SHARD], mybir.dt.uint8,
                          kind="ExternalInput").ap()
    w = nc.dram_tensor("w", [DIM, DIM], mm_dt,
                       kind="ExternalInput").ap()
    out = nc.dram_tensor("out", [DIM, SHARD], mybir.dt.float32,
                         kind="ExternalOutput").ap()

    with TileContext(nc) as tc:
        with (
            tc.tile_pool(name="wp", bufs=1) as wp,
            tc.tile_pool(name="mp", bufs=1) as mp,
            tc.tile_pool(name="xp", bufs=XP_BUFS) as xp,
            tc.tile_pool(name="xrp", bufs=XRP_BUFS) as xrp,
            tc.tile_pool(name="op", bufs=OP_BUFS) as op,
            tc.tile_pool(name="pp", bufs=PSUM_BUFS, space="PSUM") as pp,
        ):
            # preamble loads on SWDGE so the SP queue starts streaming x
            # immediately
            w_tiles = []
            for k in range(KC):
                wt = wp.tile([P, DIM], mybir.dt.float32, tag=f"w{k}")
                nc.gpsimd.dma_start(out=wt.bitcast(mm_dt),
                                    in_=w[k * P:(k + 1) * P, :])
                w_tiles.append(wt)

            # mask replicated to all 128 partitions: 16KB DRAM load, then
            # log-doubling SBUF->SBUF copies (avoids a 2MB broadcast HBM read)
            mt = mp.tile([P, SHARD], mybir.dt.uint8)
            nc.gpsimd.dma_start(out=mt[0:1, :], in_=mask)
            reps = 1
            while reps < P:
                n_copy = min(reps, P - reps)
                nc.gpsimd.dma_start(out=mt[reps:reps + n_copy, :],
                                    in_=mt[0:n_copy, :])
                reps += n_copy

            rep = (tc.For_i(0, REPEAT, 1) if REPEAT > 1
                   else contextlib.nullcontext())
            with rep:
                for b in range(NBLK):
                    _emit_block(nc, mm_dt, mybir, b, xt, out, w_tiles, mt,
                                xp, xrp, op, pp)

    nc.compile()
    return nc


def _get_compiled():
    global _compiled
    if _compiled is None:
        _compiled = _build()
    return _compiled


def kernel(x: np.ndarray, mask: np.ndarray, deletion_weight: np.ndarray,
           **_run_kwargs):
    from concourse.bass_utils import run_bass_kernel_spmd

    nc = _get_compiled()

    x = np.asarray(x, dtype=np.float32)
    w = np.ascontiguousarray(np.asarray(deletion_weight, dtype=np.float32))
    mask_u8 = np.asarray(mask).astype(np.uint8).reshape(N_TOTAL)

    in_maps = []
    for i in range(N_CORES):
        sl = slice(i * SHARD, (i + 1) * SHARD)
        in_maps.append({
            "xt": np.ascontiguousarray(x[sl].T),
            "mask": np.ascontiguousarray(mask_u8[sl]).reshape(1, SHARD),
            "w": w,
        })

    res = run_bass_kernel_spmd(nc, in_maps, core_ids=list(range(N_CORES)),
                               **_run_kwargs)

    out = np.empty((N_TOTAL, DIM), dtype=np.float32)
    for i in range(N_CORES):
        out[i * SHARD:(i + 1) * SHARD] = res.results[i]["out"].T
    if _run_kwargs:
        kernel.last_results = res
    return out
